# revision 24
# baseline (speedup 1.0000x reference)
"""AutoRec forward kernel for Trainium2, 8-core SPMD.

Math (see reference):
    agg = segment_sum(r[:,None] * v[cols], rows, m)     # sparse (m,n) @ v
    h   = sigmoid(agg + mu)                             # (M, D)
    s   = sum(h[i] * w[j])                              # global scalar over E pairs
    out = s + b[j]                                      # (E,)

Device strategy (per core, users sharded):
  Each core owns RPC = 6272 rows (users). Both heavy stages are instances of
  one primitive: "gather rows from a replicated table, weight them, and
  segment-sum into a local per-row accumulator":
    phase 1: table=v (bf16), weights=r,     rows=ij[0], cols=ij[1] -> aggT
    phase 2: table=w (f32),  weights=1.0,   rows=i,     cols=j     -> aT
          (sum_e h[i_e] * w[j_e] = sum_u h[u] . A[u],  A[u] = sum_{i_e=u} w[j_e])
  The segment-sum runs on the tensor engine: for each chunk of 128 edges the
  gathered rows form the stationary operand [128e, 128d]; a one-hot matrix
  P[e, wrow] = weight_e * (local_row_e == wrow) built on DVE is the moving
  operand; psum accumulates aggT[d, wrow] over a 64-row window. Edges are
  pre-sorted by (table-half, window) on the host so windows are contiguous,
  and the static schedule (max chunk count per group across cores) is shared
  by all cores so one SPMD program serves all 8.
  Tables are split in two 25000-row halves because dma_gather indices are
  int16. Finally h = sigmoid(aggT + mu) in one ACT op and
  s_part = sum(hT * aT) reduced on DVE; the host sums the 8 partials and
  broadcasts s + b[j] (a trivial O(E) numpy gather).
"""

import math
from dataclasses import dataclass, field

import ml_dtypes
import numpy as np

# ---------------------------------------------------------------- config

CHUNK = 128  # edges per matmul (contraction = partition dim)
IDX_WRAP = 16  # dma_gather index wrap


@dataclass
class Cfg:
    M: int = 50000          # users (rows of spmm)
    dma_scratch: int = 16384  # SWDGE descriptor carveout (bytes)
    N: int = 50000          # items (table rows)
    D: int = 128            # feature dim (must be 128)
    ncores: int = 8
    rpc: int = 6400         # rows per core (multiple of window)
    window: int = 64        # psum row-window
    half: int = 25000       # table split (int16 index limit)
    call_chunks: int = 8    # chunks per dma_gather call
    p1dt: str = "f8e4"      # value dtype of phase-1 gathers (table dtype)
    p2dt: str = "f8e4"      # value dtype of phase-2 gathers (table dtype)
    ppdt: str = "f8e4"      # dtype of the streamed one-hot P (matmul moving)
    ttb: int = 512          # block size of the final fused mul-reduce
    queues: int = 4         # SWDGE queues to round-robin gather calls over
    host_p: bool = True     # precompute one-hot P on host, stream via HWDGE
    balance: bool = True    # balanced user->(core,window) assignment
    gbufs: int = 10         # bufs in each gather pool
    pbufs: int = 12         # bufs in the P pool
    ibufs: int = 2          # bufs in the idx pool

    @property
    def nwin(self):
        return self.rpc // self.window

    def __post_init__(self):
        assert self.rpc % self.window == 0
        assert self.rpc * self.ncores >= self.M
        assert self.N <= 2 * self.half and self.half <= 32767
        assert self.D == 128


FULL = Cfg()

# ---------------------------------------------------------------- host plan


@dataclass
class PhasePlan:
    groups: list          # [(hf, win, n_chunks)] in stream order (hf-major)
    calls: list           # [(hf, chunk_start, n_chunks)]
    total_chunks: int
    # per-core packed arrays
    idx_dram: list        # [ncores] int16 [128, total_chunks*8]
    wgt_dram: list        # [ncores] [128, total_chunks]
    rl_dram: list         # [ncores] [128, total_chunks]
    p_dram: list = None   # [ncores] pdt [128, total_chunks*W] host one-hot


def _wrap_idxs(ii: np.ndarray) -> np.ndarray:
    """[n] -> [16, n/16] wrapped (t -> (t%16, t//16)); the device replicates
    to 128 partitions (8 gpsimd cores) with three doubling SBUF copies."""
    n = len(ii)
    a = ii.reshape(n // IDX_WRAP, IDX_WRAP).T
    return np.ascontiguousarray(a)


def assign_users(cfg: Cfg, loads4) -> tuple[np.ndarray, np.ndarray]:
    """Balanced user -> (core, local slot) assignment.

    loads4: [M, 4] per-user edge counts (ph1-lo, ph1-hi, ph2-lo, ph2-hi).
    Greedy vector scheduling: users heaviest-first, each to the bin (of
    8*nwin (core,window) bins with <window slots free) whose post-assign
    max-dimension load is smallest.  Keeps every (half,window) group's
    count near the global mean across cores so the shared chunk schedule
    (max over cores, ceil to 128) wastes few slots.
    Returns (core_of[M], local_of[M])."""
    nwin, W = cfg.nwin, cfg.window
    nb = cfg.ncores * nwin
    loads4 = np.asarray(loads4, np.float64)
    order = np.argsort(-loads4.sum(1), kind="stable")
    loads = np.zeros((nb, 4))
    counts = np.zeros(nb, np.int64)
    core_of = np.zeros(cfg.M, np.int64)
    local_of = np.zeros(cfg.M, np.int64)
    full_pen = np.zeros(nb)
    for u in order:
        score = (loads + loads4[u]).max(axis=1) + full_pen
        b = int(np.argmin(score))
        core_of[u] = b // nwin
        local_of[u] = (b % nwin) * W + counts[b]
        counts[b] += 1
        loads[b] += loads4[u]
        if counts[b] >= W:
            full_pen[b] = np.inf
    return core_of, local_of


def plan_phase(cfg: Cfg, rows, cols, wgts, pnp=None, core_of=None,
               local_of=None) -> PhasePlan:
    rows = np.asarray(rows, np.int64)
    cols = np.asarray(cols, np.int64)
    wgts = np.asarray(wgts, np.float32)
    nwin, ncores, Wd = cfg.nwin, cfg.ncores, cfg.window

    if core_of is not None:
        core = core_of[rows]
        local = local_of[rows]
    else:
        core = rows // cfg.rpc
        local = rows - core * cfg.rpc
    win = local // Wd
    rl = (local - win * Wd).astype(np.float32)
    hf = (cols >= cfg.half).astype(np.int64)
    idx16 = (cols - hf * cfg.half).astype(np.int16)

    key = (core * 2 + hf) * nwin + win
    counts = np.bincount(key, minlength=ncores * 2 * nwin).reshape(ncores, 2, nwin)
    nch = -(-counts.max(axis=0) // CHUNK)  # [2, nwin] ceil
    groups = []
    gbase = np.zeros((2, nwin), np.int64)
    acc = 0
    for h in range(2):
        for w in range(nwin):
            n = int(nch[h, w])
            if n == 0:
                continue
            groups.append((h, w, n))
            gbase[h, w] = acc
            acc += n
    total_chunks = acc

    # gather calls: split each half's chunk-range into spans of call_chunks
    calls = []
    cur = 0
    for h in range(2):
        nh = int(nch[h][counts.max(axis=0)[h] > 0].sum()) if nwin else 0
        # recompute exactly: chunks of half h
        nh = sum(n for (hh, _, n) in groups if hh == h)
        off = cur
        while off < cur + nh:
            n = min(cfg.call_chunks, cur + nh - off)
            calls.append((h, off, n))
            off += n
        cur += nh
    assert cur == total_chunks

    idx_l, wgt_l, rl_l, p_l = [], [], [], []
    for c in range(ncores):
        mask = core == c
        eh, ew = hf[mask], win[mask]
        erl, ei, ewgt = rl[mask], idx16[mask], wgts[mask]
        order = np.lexsort((ew, eh))
        eh, ew, erl, ei, ewgt = (a[order] for a in (eh, ew, erl, ei, ewgt))
        gid = eh * nwin + ew
        # rank within each (hf,win) run of the sorted list
        if len(gid):
            first = np.r_[True, gid[1:] != gid[:-1]]
            run_start = np.maximum.accumulate(np.where(first, np.arange(len(gid)), 0))
            rank = np.arange(len(gid)) - run_start
        else:
            rank = np.zeros(0, np.int64)
        pos = gbase[eh, ew] * CHUNK + rank
        idx_full = np.zeros(total_chunks * CHUNK, np.int16)
        wgt_full = np.zeros(total_chunks * CHUNK, np.float32)
        rl_full = np.zeros(total_chunks * CHUNK, np.float32)
        idx_full[pos] = ei
        wgt_full[pos] = ewgt
        rl_full[pos] = erl

        # wrap idx per call
        parts = []
        for (_h, c0, n) in calls:
            parts.append(_wrap_idxs(idx_full[c0 * CHUNK:(c0 + n) * CHUNK]))
        idx_l.append(np.concatenate(parts, axis=1))
        wgt_l.append(wgt_full.reshape(-1, CHUNK).T.copy())
        rl_l.append(rl_full.reshape(-1, CHUNK).T.copy())
        if pnp is not None:
            # host one-hot: P[p, chunk, x] = wgt * (rl == x), edge = chunk*128+p
            parr = np.zeros((CHUNK, total_chunks, Wd), pnp)
            epos = np.arange(total_chunks * CHUNK)
            parr[epos % CHUNK, epos // CHUNK, rl_full.astype(np.int64)] = wgt_full
            p_l.append(np.ascontiguousarray(parr.reshape(CHUNK, -1)))

    return PhasePlan(groups, calls, total_chunks, idx_l, wgt_l, rl_l,
                     p_l if pnp is not None else None)


# ---------------------------------------------------------------- device build


# fp8 tables are stored at 256B row stride (payload in the first 128 cols)
# because the SWDGE gather descriptor encodes the row stride in 256B units;
# values are pre-scaled by TAB_SCALE to clear the fp8e4m3 denormal range.
TAB_SCALE = 256.0


def raw_dma_gather(nc, out_ap, in_ap, idxs_ap, num_idxs, elem_size, elem_step,
                   queue_num, single_packet=True):
    """dma_gather with a 128-byte payload per index (elem_size_bytes need not
    be a multiple of 256; only the row stride must be).  Mirrors
    BassGpSimd.dma_gather for the transpose=False / HBM-source case."""
    import concourse.mybir as mybir

    gp = nc.gpsimd
    gp._assert_queue_num(queue_num)
    assert idxs_ap.dtype == mybir.dt.int16
    assert in_ap.dtype == out_ap.dtype
    assert in_ap.ap[0][0] == elem_step
    stride_bytes = elem_step * mybir.dt.size(in_ap.dtype)
    assert stride_bytes % 256 == 0
    stride_bytes_256 = stride_bytes // 256
    assert in_ap.ap[-1][1] == out_ap.ap[-1][1] == elem_size
    assert out_ap.ap[0][1] * out_ap.ap[1][1] == -(-num_idxs // 128) * 128
    _in_ap = gp.lower_ap_dma(in_ap, for_custom_bir_dma=True)
    _idxs_ap = gp.lower_ap(idxs_ap)
    _out_ap = gp.lower_ap(out_ap)
    return gp.add_instruction(
        mybir.InstDMAGatherAnt(
            name=gp.bass.get_next_instruction_name(),
            ins=[*_in_ap, _idxs_ap, gp.lower_val_access(gp.to_reg(num_idxs))],
            outs=[_out_ap],
            transpose=False,
            num_idxs=num_idxs,
            elem_size=elem_size,
            stride_bytes_256=stride_bytes_256,
            gen_mode=0,
            single_packet=single_packet,
            queue_num=queue_num,
            sbuf_tokens_per_rank=0,
            sbuf_free_dim_per_rank=0,
            sbuf_free_dim_pad_per_rank=0,
            sbuf_byte_offset=0,
        )
    )


def build_program(cfg: Cfg, ph1: PhasePlan, ph2: PhasePlan):
    import concourse.bacc as bacc
    import concourse.bass as bass
    import concourse.mybir as mybir
    import concourse.tile as tile

    f32 = mybir.dt.float32
    i16 = mybir.dt.int16
    DTMAP = {"f32": f32, "bf16": mybir.dt.bfloat16, "f16": mybir.dt.float16,
             "f8e4": mybir.dt.float8e4}
    p1dt, p2dt, ppdt = DTMAP[cfg.p1dt], DTMAP[cfg.p2dt], DTMAP[cfg.ppdt]
    P, Wd, RPC = 128, cfg.window, cfg.rpc
    n_hi = cfg.N - cfg.half

    nc = bacc.Bacc("TRN2", target_bir_lowering=False, debug=False,
                   dynamic_dma_scratch_size=cfg.dma_scratch,
                   num_swdge_queues=cfg.queues)

    # fp8 tables live at 256B row stride (payload = first D cols)
    tw1 = 256 if mybir.dt.size(p1dt) == 1 else cfg.D
    tw2 = 256 if mybir.dt.size(p2dt) == 1 else cfg.D
    v_lo = nc.dram_tensor("v_lo", [cfg.half, tw1], p1dt, kind="ExternalInput")
    v_hi = nc.dram_tensor("v_hi", [n_hi, tw1], p1dt, kind="ExternalInput")
    w_lo = nc.dram_tensor("w_lo", [cfg.half, tw2], p2dt, kind="ExternalInput")
    w_hi = nc.dram_tensor("w_hi", [n_hi, tw2], p2dt, kind="ExternalInput")
    mu_c = nc.dram_tensor("mu_col", [P, 1], f32, kind="ExternalInput")

    def phase_params(tag, pl: PhasePlan, pdt):
        idx = nc.dram_tensor(f"idx{tag}", [16, pl.total_chunks * 8], i16,
                             kind="ExternalInput")
        if cfg.host_p:
            pd = nc.dram_tensor(f"p{tag}", [P, pl.total_chunks * Wd], ppdt,
                                kind="ExternalInput")
            return idx, pd, None
        wgt = nc.dram_tensor(f"wgt{tag}", [P, pl.total_chunks], f32,
                             kind="ExternalInput")
        rl = nc.dram_tensor(f"rl{tag}", [P, pl.total_chunks], f32,
                            kind="ExternalInput")
        return idx, wgt, rl

    idx1, wgt1, rl1 = phase_params(1, ph1, p1dt)
    idx2, wgt2, rl2 = phase_params(2, ph2, p2dt)
    s_out = nc.dram_tensor("s_out", [P, 1], f32, kind="ExternalOutput")

    with tile.TileContext(nc) as tc:
        with (
            tc.tile_pool(name="const", bufs=1) as cpool,
            tc.tile_pool(name="idxp", bufs=cfg.ibufs) as ipool,
            tc.tile_pool(name="g1", bufs=cfg.gbufs) as g1pool,
            tc.tile_pool(name="g2", bufs=cfg.gbufs) as g2pool,
            tc.tile_pool(name="pp", bufs=cfg.pbufs) as ppool,
            tc.tile_pool(name="ev", bufs=4) as evpool,
            tc.tile_pool(name="psum", bufs=8, space="PSUM") as pspool,
        ):
            # constants
            mu_t = cpool.tile([P, 1], f32, tag="mu")
            nc.sync.dma_start(mu_t[:], mu_c[:])
            if not cfg.host_p:
                iota1 = cpool.tile([P, Wd], p1dt, tag="iota1")
                iota2 = cpool.tile([P, Wd], p2dt, tag="iota2")
                nc.gpsimd.iota(iota1[:], pattern=[[1, Wd]], base=0,
                               channel_multiplier=0,
                               allow_small_or_imprecise_dtypes=True)
                nc.gpsimd.iota(iota2[:], pattern=[[1, Wd]], base=0,
                               channel_multiplier=0,
                               allow_small_or_imprecise_dtypes=True)
            else:
                iota1 = iota2 = None

            acc1 = cpool.tile([P, RPC], f32, tag="acc1")
            acc2 = cpool.tile([P, RPC], f32, tag="acc2")
            nc.vector.memset(acc1[:], 0.0)
            nc.vector.memset(acc2[:], 0.0)

            if not cfg.host_p:
                wg1_t = cpool.tile([P, ph1.total_chunks], f32, tag="wg1")
                rl1_t = cpool.tile([P, ph1.total_chunks], f32, tag="rl1")
                wg2_t = cpool.tile([P, ph2.total_chunks], f32, tag="wg2")
                rl2_t = cpool.tile([P, ph2.total_chunks], f32, tag="rl2")
                nc.sync.dma_start(wg1_t[:], wgt1[:])
                nc.sync.dma_start(rl1_t[:], rl1[:])
                nc.sync.dma_start(wg2_t[:], wgt2[:])
                nc.sync.dma_start(rl2_t[:], rl2[:])
            else:
                wg1_t = rl1_t = wg2_t = rl2_t = None

            qcount = [0]

            def run_phase(pl: PhasePlan, tabs, idx_dram, p_dram, wg_t, rl_t,
                          acc, gpool, pdt, io_t):
                # group bookkeeping: map chunk id -> (group, first?, last?)
                chunk_group = {}
                for g, (h, w, n) in enumerate(pl.groups):
                    base = sum(nn for (_, _, nn) in pl.groups[:g])
                    for k in range(n):
                        chunk_group[base + k] = (g, w, k == 0, k == n - 1)
                # phase-wide idx tile: compact [16, n] load + 3 doubling
                # copies to replicate across the 8 gpsimd cores' partitions
                it = ipool.tile([P, pl.total_chunks * 8], i16, tag="idx")
                nc.sync.dma_start(it[0:16, :], idx_dram[:, :])
                for rep in (16, 32, 64):
                    nc.sync.dma_start(it[rep:2 * rep, :], it[0:rep, :])
                # consume calls in order, carrying the open psum group
                open_ps = None
                for (h, c0, n) in pl.calls:
                    gt = gpool.tile([P, cfg.call_chunks, cfg.D], pdt, tag="g")
                    nidx = n * CHUNK
                    islice = it[:, c0 * 8:(c0 + n) * 8]
                    if mybir.dt.size(pdt) == 1:
                        raw_dma_gather(
                            nc, gt[:, :n, :], tabs[h][:, 0:cfg.D],
                            islice, num_idxs=nidx, elem_size=cfg.D,
                            elem_step=256, queue_num=qcount[0] % cfg.queues,
                        )
                    else:
                        nc.gpsimd.dma_gather(
                            gt[:, :n, :], tabs[h][:], islice,
                            num_idxs=nidx, num_idxs_reg=nidx, elem_size=cfg.D,
                            queue_num=qcount[0] % cfg.queues,
                        )
                    qcount[0] += 1
                    if cfg.host_p:
                        pc_t = ppool.tile([P, cfg.call_chunks * Wd], ppdt,
                                          tag="pc")
                        nc.sync.dma_start(pc_t[:, : n * Wd],
                                          p_dram[:, c0 * Wd:(c0 + n) * Wd])
                    for k in range(n):
                        cid = c0 + k
                        g, w, first, last = chunk_group[cid]
                        if first:
                            open_ps = pspool.tile([P, Wd], mybir.dt.float32,
                                                  tag="ps")
                        if cfg.host_p:
                            p_ap = pc_t[:, k * Wd:(k + 1) * Wd]
                        else:
                            p_t = ppool.tile([P, Wd], pdt, tag="p")
                            nc.vector.tensor_scalar(
                                out=p_t[:], in0=io_t[:],
                                scalar1=rl_t[:, cid:cid + 1],
                                scalar2=wg_t[:, cid:cid + 1],
                                op0=mybir.AluOpType.is_equal,
                                op1=mybir.AluOpType.mult,
                            )
                            p_ap = p_t[:]
                        nc.tensor.matmul(open_ps[:], gt[:, k, :], p_ap,
                                         start=first, stop=last)
                        if last:
                            sl = acc[:, w * Wd:(w + 1) * Wd]
                            nc.vector.tensor_tensor(
                                out=sl, in0=sl, in1=open_ps[:],
                                op=mybir.AluOpType.add)

            run_phase(ph1, (v_lo, v_hi), idx1, wgt1, wg1_t, rl1_t, acc1,
                      g1pool, p1dt, iota1)
            run_phase(ph2, (w_lo, w_hi), idx2, wgt2, wg2_t, rl2_t, acc2,
                      g2pool, p2dt, iota2)

            # t = tanh((aggT + mu)/2) in place on acc1; acc1 holds ts1*aggT
            # (h = 0.5 + 0.5*t; the 0.5-mean part of s is added exactly on
            # the host, so fp8 table error only touches the small tanh term).
            # mu_c already carries mu/2.
            ts1 = TAB_SCALE if mybir.dt.size(p1dt) == 1 else 1.0
            nc.scalar.activation(acc1[:], acc1[:],
                                 mybir.ActivationFunctionType.Tanh,
                                 bias=mu_t[:, 0:1], scale=0.5 / ts1)

            # s_part[p] = sum_d sum_u h[p,u]*A[p,u]  blockwise fused mul+reduce
            nblk = math.ceil(RPC / cfg.ttb)
            s_cols = cpool.tile([P, nblk], f32, tag="scols")
            for b in range(nblk):
                lo = b * cfg.ttb
                hi = min(RPC, lo + cfg.ttb)
                tmp = evpool.tile([P, cfg.ttb], f32, tag="tmp")
                nc.vector.tensor_tensor(
                    out=tmp[:, : hi - lo],
                    in0=acc1[:, lo:hi], in1=acc2[:, lo:hi],
                    op=mybir.AluOpType.mult)
                nc.vector.tensor_reduce(
                    s_cols[:, b:b + 1], tmp[:, : hi - lo],
                    axis=mybir.AxisListType.X, op=mybir.AluOpType.add)
            s_t = cpool.tile([P, 1], f32, tag="sfin")
            nc.vector.tensor_reduce(s_t[:], s_cols[:], axis=mybir.AxisListType.X,
                                    op=mybir.AluOpType.add)
            nc.sync.dma_start(s_out[:], s_t[:])

    nc.compile()
    return nc


# ---------------------------------------------------------------- host driver


NPDT = {"f32": np.float32, "bf16": ml_dtypes.bfloat16, "f16": np.float16,
        "f8e4": ml_dtypes.float8_e4m3}


def _fmt_table(t, npdt):
    """Cast a table half for the device: fp8 tables are scaled by TAB_SCALE
    (clears the e4m3 denormal range) and padded to a 256B row stride."""
    if np.dtype(npdt).itemsize == 1:
        out = np.zeros((t.shape[0], 256), npdt)
        out[:, : t.shape[1]] = (t * TAB_SCALE).astype(npdt)
        return out
    return np.ascontiguousarray(t.astype(npdt))


def make_in_maps(cfg: Cfg, ph1: PhasePlan, ph2: PhasePlan, v, w, mu):
    p1np, p2np = NPDT[cfg.p1dt], NPDT[cfg.p2dt]
    v_lo = _fmt_table(v[:cfg.half], p1np)
    v_hi = _fmt_table(v[cfg.half:], p1np)
    w_lo = _fmt_table(w[:cfg.half], p2np)
    w_hi = _fmt_table(w[cfg.half:], p2np)
    # device activation computes tanh(in*scale + bias): bias carries mu/2
    mu_col = np.broadcast_to(0.5 * mu.reshape(-1)[:, None], (128, 1))
    mu_col = np.ascontiguousarray(mu_col.astype(np.float32))
    in_maps = []
    for c in range(cfg.ncores):
        m = {
            "v_lo": v_lo, "v_hi": v_hi, "w_lo": w_lo, "w_hi": w_hi,
            "mu_col": mu_col,
            "idx1": ph1.idx_dram[c], "idx2": ph2.idx_dram[c],
        }
        if cfg.host_p:
            m["p1"] = ph1.p_dram[c]
            m["p2"] = ph2.p_dram[c]
        else:
            m.update({"wgt1": ph1.wgt_dram[c], "rl1": ph1.rl_dram[c],
                      "wgt2": ph2.wgt_dram[c], "rl2": ph2.rl_dram[c]})
        in_maps.append(m)
    return in_maps


def prepare(cfg: Cfg, ij, r, i, j):
    pnp = NPDT[cfg.ppdt] if cfg.host_p else None
    core_of = local_of = None
    if cfg.balance:
        rows1 = np.asarray(ij[0], np.int64)
        hf1 = np.asarray(ij[1], np.int64) >= cfg.half
        rows2 = np.asarray(i, np.int64)
        hf2 = np.asarray(j, np.int64) >= cfg.half
        loads4 = np.stack(
            [np.bincount(rows1[~hf1], minlength=cfg.M),
             np.bincount(rows1[hf1], minlength=cfg.M),
             np.bincount(rows2[~hf2], minlength=cfg.M),
             np.bincount(rows2[hf2], minlength=cfg.M)], axis=1)
        core_of, local_of = assign_users(cfg, loads4)
    ph1 = plan_phase(cfg, ij[0], ij[1], r, pnp, core_of, local_of)
    ph2 = plan_phase(cfg, i, j, np.ones(len(i), np.float32), pnp, core_of,
                     local_of)
    return ph1, ph2


_prog_cache = {}


def kernel(ij, r, m, i, j, v, mu, w, b, cfg: Cfg = FULL, _return_parts=False,
           _run_kwargs=None):
    from concourse.bass_utils import run_bass_kernel_spmd

    ij = np.asarray(ij)
    r = np.asarray(r, np.float32)
    i = np.asarray(i)
    j = np.asarray(j)
    v = np.asarray(v, np.float32)
    w = np.asarray(w, np.float32)
    mu = np.asarray(mu, np.float32)
    b = np.asarray(b, np.float32)
    assert int(m) == cfg.M

    ph1, ph2 = prepare(cfg, ij, r, i, j)
    key = (cfg.M, cfg.N, ph1.total_chunks, ph2.total_chunks,
           tuple(ph1.groups), tuple(ph2.groups))
    if key not in _prog_cache:
        _prog_cache.clear()
        _prog_cache[key] = build_program(cfg, ph1, ph2)
    nc = _prog_cache[key]

    in_maps = make_in_maps(cfg, ph1, ph2, v, w, mu)
    res = run_bass_kernel_spmd(nc, in_maps, list(range(cfg.ncores)),
                               **(_run_kwargs or {}))
    parts = [res.results[c]["s_out"] for c in range(cfg.ncores)]
    # s = sum_e h[i]·w[j] with h = 0.5 + 0.5*tanh((agg+mu)/2):
    #   0.5*T (mean part, exact in f64 on host; T = sum_e sum_d w[j_e,d])
    # + 0.5*sum_u tanh_u·A_u (device partials; acc2 holds ts2*A)
    ts2 = TAB_SCALE if np.dtype(NPDT[cfg.p2dt]).itemsize == 1 else 1.0
    cnt_j = np.bincount(np.asarray(j, np.int64), minlength=cfg.N)
    T = float(cnt_j.astype(np.float64) @ w.astype(np.float64).sum(axis=1))
    s_dev = sum(np.asarray(p, np.float64).sum() for p in parts)
    s = np.float32(0.5 * T + 0.5 * s_dev / ts2)
    out = s + b[j]
    if _return_parts:
        return out, res
    return out



# revision 26
# speedup vs baseline: 2.0245x; 2.0245x over previous
"""AutoRec forward kernel for Trainium2, 8-core SPMD.

Math (see reference):
    agg = segment_sum(r[:,None] * v[cols], rows, m)     # sparse (m,n) @ v
    h   = sigmoid(agg + mu)                             # (M, D)
    s   = sum(h[i] * w[j])                              # global scalar over E pairs
    out = s + b[j]                                      # (E,)

Device strategy (per core, users sharded):
  Users are dealt onto the 8 cores' RPC=6400 row slots by a 4-D greedy
  balancer (assign_users) so every (table-half, psum-window) group has a
  near-equal edge count on all cores; one SPMD program (schedule = max
  chunk count per group) then serves all 8 cores with ~2.4% pad.
  Both heavy stages are instances of one primitive — "gather rows from a
  replicated table, weight them, segment-sum into a per-row accumulator":
    phase 1: table=v (fp8), weights=r,   rows=ij[0], cols=ij[1] -> aggT
    phase 2: table=w (fp8), weights=1.0, rows=i,     cols=j     -> A^T
          (sum_e h[i_e]*w[j_e] = sum_u h[u].A[u], A[u] = sum_{i_e=u} w[j_e])
  Tables are fp8e4m3 scaled by TAB_SCALE (clears the denormal range),
  stored at 256B row stride, and gathered 128B/row by SWDGE (raw
  InstDMAGatherAnt: elem_size_bytes%256 only constrains the row stride);
  1024 idxs per call is a hard ucode cap (2048 fails in single_packet
  mode).  Each 128-edge chunk lands as the matmul stationary [128e,128d];
  the moving operand is a host-built fp8 one-hot P[e,wrow]=wgt_e*
  (local_row_e==wrow) streamed via HWDGE; psum accumulates aggT[d,wrow]
  over 128-row windows.  Two 25000-row table halves keep idxs in int16.
  Precision rescue: the device computes t = tanh((agg+mu)/2) (one ACT op)
  and s_part = sum(t^T ⊙ A^T) on DVE; with h = 0.5 + 0.5*tanh(.), the
  host adds the mean term 0.5*T (T = sum_n cnt_j[n]*rowsum(w[n]), exact
  f64) so fp8 quantization of w only touches the small tanh term:
      out = 0.5*T + 0.5*sum(parts)/TAB_SCALE + b[j].
"""

import math
from dataclasses import dataclass, field

import ml_dtypes
import numpy as np

# ---------------------------------------------------------------- config

CHUNK = 128  # edges per matmul (contraction = partition dim)
IDX_WRAP = 16  # dma_gather index wrap


@dataclass
class Cfg:
    M: int = 50000          # users (rows of spmm)
    dma_scratch: int = 16384  # SWDGE descriptor carveout (bytes)
    N: int = 50000          # items (table rows)
    D: int = 128            # feature dim (must be 128)
    ncores: int = 8
    rpc: int = 6400         # rows per core (multiple of window)
    window: int = 128       # psum row-window
    half: int = 25000       # table split (int16 index limit)
    call_chunks: int = 8    # chunks per dma_gather call
    p1dt: str = "f8e4"      # value dtype of phase-1 gathers (table dtype)
    p2dt: str = "f8e4"      # value dtype of phase-2 gathers (table dtype)
    ppdt: str = "f8e4"      # dtype of the streamed one-hot P (matmul moving)
    ttb: int = 512          # block size of the final fused mul-reduce
    queues: int = 4         # SWDGE queues to round-robin gather calls over
    host_p: bool = True     # precompute one-hot P on host, stream via HWDGE
    balance: bool = True    # balanced user->(core,window) assignment
    gbufs: int = 8          # bufs in each gather pool
    pbufs: int = 10         # bufs in the P pool
    ibufs: int = 10         # bufs in the idx pool

    @property
    def nwin(self):
        return self.rpc // self.window

    def __post_init__(self):
        assert self.rpc % self.window == 0
        assert self.rpc * self.ncores >= self.M
        assert self.N <= 2 * self.half and self.half <= 32767
        assert self.D == 128


FULL = Cfg()

# ---------------------------------------------------------------- host plan


@dataclass
class PhasePlan:
    groups: list          # [(hf, win, n_chunks)] in stream order (hf-major)
    calls: list           # [(hf, chunk_start, n_chunks)]
    total_chunks: int
    # per-core packed arrays
    idx_dram: list        # [ncores] int16 [128, total_chunks*8]
    wgt_dram: list        # [ncores] [128, total_chunks]
    rl_dram: list         # [ncores] [128, total_chunks]
    p_dram: list = None   # [ncores] pdt [128, total_chunks*W] host one-hot


def _wrap_idxs(ii: np.ndarray) -> np.ndarray:
    """[n] -> [128, n/16] wrapped (t -> (t%16, t//16)), replicated x8."""
    n = len(ii)
    a = ii.reshape(n // IDX_WRAP, IDX_WRAP).T
    return np.tile(a, (8, 1))


def assign_users(cfg: Cfg, loads4) -> tuple[np.ndarray, np.ndarray]:
    """Balanced user -> (core, local slot) assignment.

    loads4: [M, 4] per-user edge counts (ph1-lo, ph1-hi, ph2-lo, ph2-hi).
    Greedy vector scheduling: users heaviest-first, each to the bin (of
    8*nwin (core,window) bins with <window slots free) whose post-assign
    max-dimension load is smallest.  Keeps every (half,window) group's
    count near the global mean across cores so the shared chunk schedule
    (max over cores, ceil to 128) wastes few slots.
    Returns (core_of[M], local_of[M])."""
    nwin, W = cfg.nwin, cfg.window
    nb = cfg.ncores * nwin
    loads4 = np.asarray(loads4, np.float64)
    order = np.argsort(-loads4.sum(1), kind="stable")
    loads = np.zeros((nb, 4))
    counts = np.zeros(nb, np.int64)
    core_of = np.zeros(cfg.M, np.int64)
    local_of = np.zeros(cfg.M, np.int64)
    full_pen = np.zeros(nb)
    for u in order:
        score = (loads + loads4[u]).max(axis=1) + full_pen
        b = int(np.argmin(score))
        core_of[u] = b // nwin
        local_of[u] = (b % nwin) * W + counts[b]
        counts[b] += 1
        loads[b] += loads4[u]
        if counts[b] >= W:
            full_pen[b] = np.inf
    return core_of, local_of


def plan_phase(cfg: Cfg, rows, cols, wgts, pnp=None, core_of=None,
               local_of=None) -> PhasePlan:
    rows = np.asarray(rows, np.int64)
    cols = np.asarray(cols, np.int64)
    wgts = np.asarray(wgts, np.float32)
    nwin, ncores, Wd = cfg.nwin, cfg.ncores, cfg.window

    if core_of is not None:
        core = core_of[rows]
        local = local_of[rows]
    else:
        core = rows // cfg.rpc
        local = rows - core * cfg.rpc
    win = local // Wd
    rl = (local - win * Wd).astype(np.float32)
    hf = (cols >= cfg.half).astype(np.int64)
    idx16 = (cols - hf * cfg.half).astype(np.int16)

    key = (core * 2 + hf) * nwin + win
    counts = np.bincount(key, minlength=ncores * 2 * nwin).reshape(ncores, 2, nwin)
    nch = -(-counts.max(axis=0) // CHUNK)  # [2, nwin] ceil
    groups = []
    gbase = np.zeros((2, nwin), np.int64)
    acc = 0
    for h in range(2):
        for w in range(nwin):
            n = int(nch[h, w])
            if n == 0:
                continue
            groups.append((h, w, n))
            gbase[h, w] = acc
            acc += n
    total_chunks = acc

    # gather calls: split each half's chunk-range into spans of call_chunks
    calls = []
    cur = 0
    for h in range(2):
        nh = int(nch[h][counts.max(axis=0)[h] > 0].sum()) if nwin else 0
        # recompute exactly: chunks of half h
        nh = sum(n for (hh, _, n) in groups if hh == h)
        off = cur
        while off < cur + nh:
            n = min(cfg.call_chunks, cur + nh - off)
            calls.append((h, off, n))
            off += n
        cur += nh
    assert cur == total_chunks

    idx_l, wgt_l, rl_l, p_l = [], [], [], []
    for c in range(ncores):
        mask = core == c
        eh, ew = hf[mask], win[mask]
        erl, ei, ewgt = rl[mask], idx16[mask], wgts[mask]
        order = np.lexsort((ew, eh))
        eh, ew, erl, ei, ewgt = (a[order] for a in (eh, ew, erl, ei, ewgt))
        gid = eh * nwin + ew
        # rank within each (hf,win) run of the sorted list
        if len(gid):
            first = np.r_[True, gid[1:] != gid[:-1]]
            run_start = np.maximum.accumulate(np.where(first, np.arange(len(gid)), 0))
            rank = np.arange(len(gid)) - run_start
        else:
            rank = np.zeros(0, np.int64)
        pos = gbase[eh, ew] * CHUNK + rank
        idx_full = np.zeros(total_chunks * CHUNK, np.int16)
        wgt_full = np.zeros(total_chunks * CHUNK, np.float32)
        rl_full = np.zeros(total_chunks * CHUNK, np.float32)
        idx_full[pos] = ei
        wgt_full[pos] = ewgt
        rl_full[pos] = erl

        # wrap idx per call
        parts = []
        for (_h, c0, n) in calls:
            parts.append(_wrap_idxs(idx_full[c0 * CHUNK:(c0 + n) * CHUNK]))
        idx_l.append(np.concatenate(parts, axis=1))
        wgt_l.append(wgt_full.reshape(-1, CHUNK).T.copy())
        rl_l.append(rl_full.reshape(-1, CHUNK).T.copy())
        if pnp is not None:
            # host one-hot: P[p, chunk, x] = wgt * (rl == x), edge = chunk*128+p
            parr = np.zeros((CHUNK, total_chunks, Wd), pnp)
            epos = np.arange(total_chunks * CHUNK)
            parr[epos % CHUNK, epos // CHUNK, rl_full.astype(np.int64)] = wgt_full
            p_l.append(np.ascontiguousarray(parr.reshape(CHUNK, -1)))

    return PhasePlan(groups, calls, total_chunks, idx_l, wgt_l, rl_l,
                     p_l if pnp is not None else None)


# ---------------------------------------------------------------- device build


# fp8 tables are stored at 256B row stride (payload in the first 128 cols)
# because the SWDGE gather descriptor encodes the row stride in 256B units;
# values are pre-scaled by TAB_SCALE to clear the fp8e4m3 denormal range.
TAB_SCALE = 256.0


def raw_dma_gather(nc, out_ap, in_ap, idxs_ap, num_idxs, elem_size, elem_step,
                   queue_num, single_packet=True):
    """dma_gather with a 128-byte payload per index (elem_size_bytes need not
    be a multiple of 256; only the row stride must be).  Mirrors
    BassGpSimd.dma_gather for the transpose=False / HBM-source case."""
    import concourse.mybir as mybir

    gp = nc.gpsimd
    gp._assert_queue_num(queue_num)
    assert idxs_ap.dtype == mybir.dt.int16
    assert in_ap.dtype == out_ap.dtype
    assert in_ap.ap[0][0] == elem_step
    stride_bytes = elem_step * mybir.dt.size(in_ap.dtype)
    assert stride_bytes % 256 == 0
    stride_bytes_256 = stride_bytes // 256
    assert in_ap.ap[-1][1] == out_ap.ap[-1][1] == elem_size
    assert out_ap.ap[0][1] * out_ap.ap[1][1] == -(-num_idxs // 128) * 128
    _in_ap = gp.lower_ap_dma(in_ap, for_custom_bir_dma=True)
    _idxs_ap = gp.lower_ap(idxs_ap)
    _out_ap = gp.lower_ap(out_ap)
    return gp.add_instruction(
        mybir.InstDMAGatherAnt(
            name=gp.bass.get_next_instruction_name(),
            ins=[*_in_ap, _idxs_ap, gp.lower_val_access(gp.to_reg(num_idxs))],
            outs=[_out_ap],
            transpose=False,
            num_idxs=num_idxs,
            elem_size=elem_size,
            stride_bytes_256=stride_bytes_256,
            gen_mode=0,
            single_packet=single_packet,
            queue_num=queue_num,
            sbuf_tokens_per_rank=0,
            sbuf_free_dim_per_rank=0,
            sbuf_free_dim_pad_per_rank=0,
            sbuf_byte_offset=0,
        )
    )


def build_program(cfg: Cfg, ph1: PhasePlan, ph2: PhasePlan):
    import concourse.bacc as bacc
    import concourse.bass as bass
    import concourse.mybir as mybir
    import concourse.tile as tile

    f32 = mybir.dt.float32
    i16 = mybir.dt.int16
    DTMAP = {"f32": f32, "bf16": mybir.dt.bfloat16, "f16": mybir.dt.float16,
             "f8e4": mybir.dt.float8e4}
    p1dt, p2dt, ppdt = DTMAP[cfg.p1dt], DTMAP[cfg.p2dt], DTMAP[cfg.ppdt]
    P, Wd, RPC = 128, cfg.window, cfg.rpc
    n_hi = cfg.N - cfg.half

    nc = bacc.Bacc("TRN2", target_bir_lowering=False, debug=False,
                   dynamic_dma_scratch_size=cfg.dma_scratch,
                   num_swdge_queues=cfg.queues)

    # fp8 tables live at 256B row stride (payload = first D cols)
    tw1 = 256 if mybir.dt.size(p1dt) == 1 else cfg.D
    tw2 = 256 if mybir.dt.size(p2dt) == 1 else cfg.D
    v_lo = nc.dram_tensor("v_lo", [cfg.half, tw1], p1dt, kind="ExternalInput")
    v_hi = nc.dram_tensor("v_hi", [n_hi, tw1], p1dt, kind="ExternalInput")
    w_lo = nc.dram_tensor("w_lo", [cfg.half, tw2], p2dt, kind="ExternalInput")
    w_hi = nc.dram_tensor("w_hi", [n_hi, tw2], p2dt, kind="ExternalInput")
    mu_c = nc.dram_tensor("mu_col", [P, 1], f32, kind="ExternalInput")

    def phase_params(tag, pl: PhasePlan, pdt):
        idx = nc.dram_tensor(f"idx{tag}", [P, pl.total_chunks * 8], i16,
                             kind="ExternalInput")
        if cfg.host_p:
            pd = nc.dram_tensor(f"p{tag}", [P, pl.total_chunks * Wd], ppdt,
                                kind="ExternalInput")
            return idx, pd, None
        wgt = nc.dram_tensor(f"wgt{tag}", [P, pl.total_chunks], f32,
                             kind="ExternalInput")
        rl = nc.dram_tensor(f"rl{tag}", [P, pl.total_chunks], f32,
                            kind="ExternalInput")
        return idx, wgt, rl

    idx1, wgt1, rl1 = phase_params(1, ph1, p1dt)
    idx2, wgt2, rl2 = phase_params(2, ph2, p2dt)
    s_out = nc.dram_tensor("s_out", [P, 1], f32, kind="ExternalOutput")

    with tile.TileContext(nc) as tc:
        with (
            tc.tile_pool(name="const", bufs=1) as cpool,
            tc.tile_pool(name="idxp", bufs=cfg.ibufs) as ipool,
            tc.tile_pool(name="g1", bufs=cfg.gbufs) as g1pool,
            tc.tile_pool(name="g2", bufs=cfg.gbufs) as g2pool,
            tc.tile_pool(name="pp", bufs=cfg.pbufs) as ppool,
            tc.tile_pool(name="ev", bufs=4) as evpool,
            tc.tile_pool(name="psum", bufs=8, space="PSUM") as pspool,
        ):
            # constants
            mu_t = cpool.tile([P, 1], f32, tag="mu")
            nc.sync.dma_start(mu_t[:], mu_c[:])
            if not cfg.host_p:
                iota1 = cpool.tile([P, Wd], p1dt, tag="iota1")
                iota2 = cpool.tile([P, Wd], p2dt, tag="iota2")
                nc.gpsimd.iota(iota1[:], pattern=[[1, Wd]], base=0,
                               channel_multiplier=0,
                               allow_small_or_imprecise_dtypes=True)
                nc.gpsimd.iota(iota2[:], pattern=[[1, Wd]], base=0,
                               channel_multiplier=0,
                               allow_small_or_imprecise_dtypes=True)
            else:
                iota1 = iota2 = None

            acc1 = cpool.tile([P, RPC], f32, tag="acc1")
            acc2 = cpool.tile([P, RPC], f32, tag="acc2")
            nc.vector.memset(acc1[:], 0.0)
            nc.vector.memset(acc2[:], 0.0)

            if not cfg.host_p:
                wg1_t = cpool.tile([P, ph1.total_chunks], f32, tag="wg1")
                rl1_t = cpool.tile([P, ph1.total_chunks], f32, tag="rl1")
                wg2_t = cpool.tile([P, ph2.total_chunks], f32, tag="wg2")
                rl2_t = cpool.tile([P, ph2.total_chunks], f32, tag="rl2")
                nc.sync.dma_start(wg1_t[:], wgt1[:])
                nc.sync.dma_start(rl1_t[:], rl1[:])
                nc.sync.dma_start(wg2_t[:], wgt2[:])
                nc.sync.dma_start(rl2_t[:], rl2[:])
            else:
                wg1_t = rl1_t = wg2_t = rl2_t = None

            qcount = [0]

            def run_phase(pl: PhasePlan, tabs, idx_dram, p_dram, wg_t, rl_t,
                          acc, gpool, pdt, io_t):
                # group bookkeeping: map chunk id -> (group, first?, last?)
                chunk_group = {}
                for g, (h, w, n) in enumerate(pl.groups):
                    base = sum(nn for (_, _, nn) in pl.groups[:g])
                    for k in range(n):
                        chunk_group[base + k] = (g, w, k == 0, k == n - 1)
                # consume calls in order, carrying the open psum group
                open_ps = None
                for (h, c0, n) in pl.calls:
                    it = ipool.tile([P, cfg.call_chunks * 8], i16, tag="idx")
                    nc.sync.dma_start(it[:, : n * 8],
                                      idx_dram[:, c0 * 8:(c0 + n) * 8])
                    gt = gpool.tile([P, cfg.call_chunks, cfg.D], pdt, tag="g")
                    nidx = n * CHUNK
                    islice = it[:, : n * 8]
                    if mybir.dt.size(pdt) == 1:
                        raw_dma_gather(
                            nc, gt[:, :n, :], tabs[h][:, 0:cfg.D],
                            islice, num_idxs=nidx, elem_size=cfg.D,
                            elem_step=256, queue_num=qcount[0] % cfg.queues,
                        )
                    else:
                        nc.gpsimd.dma_gather(
                            gt[:, :n, :], tabs[h][:], islice,
                            num_idxs=nidx, num_idxs_reg=nidx, elem_size=cfg.D,
                            queue_num=qcount[0] % cfg.queues,
                        )
                    qcount[0] += 1
                    if cfg.host_p:
                        pc_t = ppool.tile([P, cfg.call_chunks * Wd], ppdt,
                                          tag="pc")
                        nc.sync.dma_start(pc_t[:, : n * Wd],
                                          p_dram[:, c0 * Wd:(c0 + n) * Wd])
                    for k in range(n):
                        cid = c0 + k
                        g, w, first, last = chunk_group[cid]
                        if first:
                            open_ps = pspool.tile([P, Wd], mybir.dt.float32,
                                                  tag="ps")
                        if cfg.host_p:
                            p_ap = pc_t[:, k * Wd:(k + 1) * Wd]
                        else:
                            p_t = ppool.tile([P, Wd], pdt, tag="p")
                            nc.vector.tensor_scalar(
                                out=p_t[:], in0=io_t[:],
                                scalar1=rl_t[:, cid:cid + 1],
                                scalar2=wg_t[:, cid:cid + 1],
                                op0=mybir.AluOpType.is_equal,
                                op1=mybir.AluOpType.mult,
                            )
                            p_ap = p_t[:]
                        nc.tensor.matmul(open_ps[:], gt[:, k, :], p_ap,
                                         start=first, stop=last)
                        if last:
                            sl = acc[:, w * Wd:(w + 1) * Wd]
                            nc.vector.tensor_tensor(
                                out=sl, in0=sl, in1=open_ps[:],
                                op=mybir.AluOpType.add)

            run_phase(ph1, (v_lo, v_hi), idx1, wgt1, wg1_t, rl1_t, acc1,
                      g1pool, p1dt, iota1)
            run_phase(ph2, (w_lo, w_hi), idx2, wgt2, wg2_t, rl2_t, acc2,
                      g2pool, p2dt, iota2)

            # t = tanh((aggT + mu)/2) in place on acc1; acc1 holds ts1*aggT
            # (h = 0.5 + 0.5*t; the 0.5-mean part of s is added exactly on
            # the host, so fp8 table error only touches the small tanh term).
            # mu_c already carries mu/2.
            ts1 = TAB_SCALE if mybir.dt.size(p1dt) == 1 else 1.0
            nc.scalar.activation(acc1[:], acc1[:],
                                 mybir.ActivationFunctionType.Tanh,
                                 bias=mu_t[:, 0:1], scale=0.5 / ts1)

            # s_part[p] = sum_d sum_u h[p,u]*A[p,u]  blockwise fused mul+reduce
            nblk = math.ceil(RPC / cfg.ttb)
            s_cols = cpool.tile([P, nblk], f32, tag="scols")
            for b in range(nblk):
                lo = b * cfg.ttb
                hi = min(RPC, lo + cfg.ttb)
                tmp = evpool.tile([P, cfg.ttb], f32, tag="tmp")
                nc.vector.tensor_tensor(
                    out=tmp[:, : hi - lo],
                    in0=acc1[:, lo:hi], in1=acc2[:, lo:hi],
                    op=mybir.AluOpType.mult)
                nc.vector.tensor_reduce(
                    s_cols[:, b:b + 1], tmp[:, : hi - lo],
                    axis=mybir.AxisListType.X, op=mybir.AluOpType.add)
            s_t = cpool.tile([P, 1], f32, tag="sfin")
            nc.vector.tensor_reduce(s_t[:], s_cols[:], axis=mybir.AxisListType.X,
                                    op=mybir.AluOpType.add)
            nc.sync.dma_start(s_out[:], s_t[:])

    nc.compile()
    return nc


# ---------------------------------------------------------------- host driver


NPDT = {"f32": np.float32, "bf16": ml_dtypes.bfloat16, "f16": np.float16,
        "f8e4": ml_dtypes.float8_e4m3}


def _fmt_table(t, npdt):
    """Cast a table half for the device: fp8 tables are scaled by TAB_SCALE
    (clears the e4m3 denormal range) and padded to a 256B row stride."""
    if np.dtype(npdt).itemsize == 1:
        out = np.zeros((t.shape[0], 256), npdt)
        out[:, : t.shape[1]] = (t * TAB_SCALE).astype(npdt)
        return out
    return np.ascontiguousarray(t.astype(npdt))


def make_in_maps(cfg: Cfg, ph1: PhasePlan, ph2: PhasePlan, v, w, mu):
    p1np, p2np = NPDT[cfg.p1dt], NPDT[cfg.p2dt]
    v_lo = _fmt_table(v[:cfg.half], p1np)
    v_hi = _fmt_table(v[cfg.half:], p1np)
    w_lo = _fmt_table(w[:cfg.half], p2np)
    w_hi = _fmt_table(w[cfg.half:], p2np)
    # device activation computes tanh(in*scale + bias): bias carries mu/2
    mu_col = np.broadcast_to(0.5 * mu.reshape(-1)[:, None], (128, 1))
    mu_col = np.ascontiguousarray(mu_col.astype(np.float32))
    in_maps = []
    for c in range(cfg.ncores):
        m = {
            "v_lo": v_lo, "v_hi": v_hi, "w_lo": w_lo, "w_hi": w_hi,
            "mu_col": mu_col,
            "idx1": ph1.idx_dram[c], "idx2": ph2.idx_dram[c],
        }
        if cfg.host_p:
            m["p1"] = ph1.p_dram[c]
            m["p2"] = ph2.p_dram[c]
        else:
            m.update({"wgt1": ph1.wgt_dram[c], "rl1": ph1.rl_dram[c],
                      "wgt2": ph2.wgt_dram[c], "rl2": ph2.rl_dram[c]})
        in_maps.append(m)
    return in_maps


def prepare(cfg: Cfg, ij, r, i, j):
    pnp = NPDT[cfg.ppdt] if cfg.host_p else None
    core_of = local_of = None
    if cfg.balance:
        rows1 = np.asarray(ij[0], np.int64)
        hf1 = np.asarray(ij[1], np.int64) >= cfg.half
        rows2 = np.asarray(i, np.int64)
        hf2 = np.asarray(j, np.int64) >= cfg.half
        loads4 = np.stack(
            [np.bincount(rows1[~hf1], minlength=cfg.M),
             np.bincount(rows1[hf1], minlength=cfg.M),
             np.bincount(rows2[~hf2], minlength=cfg.M),
             np.bincount(rows2[hf2], minlength=cfg.M)], axis=1)
        core_of, local_of = assign_users(cfg, loads4)
    ph1 = plan_phase(cfg, ij[0], ij[1], r, pnp, core_of, local_of)
    ph2 = plan_phase(cfg, i, j, np.ones(len(i), np.float32), pnp, core_of,
                     local_of)
    return ph1, ph2


_prog_cache = {}


def kernel(ij, r, m, i, j, v, mu, w, b, cfg: Cfg = FULL, _return_parts=False,
           _run_kwargs=None):
    from concourse.bass_utils import run_bass_kernel_spmd

    ij = np.asarray(ij)
    r = np.asarray(r, np.float32)
    i = np.asarray(i)
    j = np.asarray(j)
    v = np.asarray(v, np.float32)
    w = np.asarray(w, np.float32)
    mu = np.asarray(mu, np.float32)
    b = np.asarray(b, np.float32)
    assert int(m) == cfg.M

    ph1, ph2 = prepare(cfg, ij, r, i, j)
    key = (cfg.M, cfg.N, ph1.total_chunks, ph2.total_chunks,
           tuple(ph1.groups), tuple(ph2.groups))
    if key not in _prog_cache:
        _prog_cache.clear()
        _prog_cache[key] = build_program(cfg, ph1, ph2)
    nc = _prog_cache[key]

    in_maps = make_in_maps(cfg, ph1, ph2, v, w, mu)
    res = run_bass_kernel_spmd(nc, in_maps, list(range(cfg.ncores)),
                               **(_run_kwargs or {}))
    parts = [res.results[c]["s_out"] for c in range(cfg.ncores)]
    # s = sum_e h[i]·w[j] with h = 0.5 + 0.5*tanh((agg+mu)/2):
    #   0.5*T (mean part, exact in f64 on host; T = sum_e sum_d w[j_e,d])
    # + 0.5*sum_u tanh_u·A_u (device partials; acc2 holds ts2*A)
    ts2 = TAB_SCALE if np.dtype(NPDT[cfg.p2dt]).itemsize == 1 else 1.0
    cnt_j = np.bincount(np.asarray(j, np.int64), minlength=cfg.N)
    T = float(cnt_j.astype(np.float64) @ w.astype(np.float64).sum(axis=1))
    s_dev = sum(np.asarray(p, np.float64).sum() for p in parts)
    s = np.float32(0.5 * T + 0.5 * s_dev / ts2)
    out = s + b[j]
    if _return_parts:
        return out, res
    return out

